# revision 30
# baseline (speedup 1.0000x reference)
"""Expert-parallel MoE (top-k routing + SwiGLU experts) for 8 Trainium2 cores.

Strategy (balanced slot-packing, slot-major, deadline-ordered prologue)
-----------------------------------------------------------------------
- Host computes the (tiny) gate: logits = x @ gate_w (+ noise * noise_weight),
  top-k selection, sparse softmax weights.  0.03% of total FLOPs.
- Load balancing: instead of one expert per core padded to the max expert's
  token count (C=2176 for these counts), every core gets 4 weight-SLOTS of
  sizes (512, 544, 504, 496) -- capacity 2056 vs. the perfect 2048.  A slot
  holds tokens of a single expert; a small DP assigns each expert a multiset
  of slot instances across cores so all 16384 (token, expert) pairs fit with
  minimal padding.  Each slot's weights are streamed independently (the
  uniform SPMD program cannot dedup same-expert slots), ~96 MB/core of HBM
  reads -- fine: the 3 DMA queues burst at 150-190 GB/s each and sit mostly
  idle.
- Slot-major loop: for slot s: for h-group g: stream w1/w2/wp(s, g),
  accumulate out_acc(s) over g; at g==7 the epilogue streams the slot's
  output DMA immediately, so the kernel tail is just the last slot's
  last-dm epilogue + drain.  The final window is un-pipelined and uses the
  fold-gw epilogue (gate folded into ht and pre-scaled into oacc on the
  Pool engine) so the tail after the last matmul is one DVE add per dm.
- w1/w2 SBUF layout is hj-major ([128, (hj, k, 128)]) and wp is dm-major
  ([128, (dm, hk, 128)]); x is chunk-major per slot.  The first h-chain
  then only needs x-slot0 + one 256KB weight block; the prologue streams
  ~6MB in consumption-deadline order across all three DMA rings while the
  PE computes behind it.
- Device kernel math (tokens on the free axis; bf16 matmul inputs, f32
  PSUM accumulation):
    hT[128h, tok] = (w1g.T @ xT + b1) * silu(w2g.T @ xT + b2)   (bf16)
    out_acc[128d, tok] += wpg.T @ hT          (PSUM acc over the 512 h)
  The 544-slot runs as 2x272-wide matmuls (PSUM bank is 512 f32 wide;
  >=128-wide moving keeps the stationary-weight load hidden).
- Software pipelining across (slot, g) windows: window w's dm-phase (psB
  chains) is emitted after window w+1's h-phase so the PE FIFO never waits
  on the cross-engine silu/STT chain.  Mid-kernel epilogue gate-multiplies
  go to the Pool engine (GPSIMD cannot read PSUM; adds stay on DVE).

Measured (core 0 NTFF): 695.6us vs 746.1us for the one-expert-per-core
baseline; PE busy 674us at ~96% of bf16 peak, HBM reads ~106MB/core.
"""

import sys
import numpy as np

sys.path.insert(0, "/opt/trn_rl_repo")

D = 1024
H = 4096
E = 8
KD = D // 128          # 8 k-tiles over D
G = 8                  # h-groups
HJ = 4                 # 128-row h-tiles per group (G*HJ*128 == H)
NSLOT = 4
WARMUP_MMS = 22   # dep-free PE warmup bridging the prologue DMA fill so
                  # the HAM clock never drops to 4/8 before the first
                  # real chains (~13us in)

_NC_CACHE = {}


def _chunks(sl):
    """Split a slot of sl tokens into matmul-width chunks (<=512, >=128)."""
    if sl <= 512:
        return [(0, sl)]
    half = (sl + 1) // 2
    half = ((half + 15) // 16) * 16
    return [(0, half), (half, sl - half)]


def _build(sizes):
    import concourse.mybir as mybir
    import concourse.tile as tile
    from concourse import bacc

    f32 = mybir.dt.float32
    bf16 = mybir.dt.bfloat16
    ACT = mybir.ActivationFunctionType
    ALU = mybir.AluOpType

    CAP = sum(sizes)
    offs = [sum(sizes[:i]) for i in range(NSLOT)]

    nc = bacc.Bacc()
    # all inputs pre-arranged on the host into SBUF tile layout
    xeT = nc.dram_tensor("xeT", [128, KD * CAP], bf16, kind="ExternalInput")
    w1 = nc.dram_tensor("w1", [NSLOT, G, 128, HJ * KD * 128], bf16,
                        kind="ExternalInput")
    w2 = nc.dram_tensor("w2", [NSLOT, G, 128, HJ * KD * 128], bf16,
                        kind="ExternalInput")
    wp = nc.dram_tensor("wp", [NSLOT, G, 128, KD * HJ * 128], bf16,
                        kind="ExternalInput")
    b1 = nc.dram_tensor("b1", [NSLOT, 128, G * HJ], f32, kind="ExternalInput")
    b2 = nc.dram_tensor("b2", [NSLOT, 128, G * HJ], f32, kind="ExternalInput")
    bp = nc.dram_tensor("bp", [NSLOT, 128, KD], f32, kind="ExternalInput")
    gwb = nc.dram_tensor("gwb", [128, CAP], f32, kind="ExternalInput")
    outT = nc.dram_tensor("outT", [D, CAP], bf16, kind="ExternalOutput")

    with tile.TileContext(nc) as tc:
        with (
            tc.tile_pool(name="pwu", bufs=1) as pwu,
            tc.tile_pool(name="pw12", bufs=2) as pw12,
            tc.tile_pool(name="pwp", bufs=2) as pwp,
            tc.tile_pool(name="px", bufs=1) as px,
            tc.tile_pool(name="pht", bufs=2) as pht,
            tc.tile_pool(name="ps2", bufs=3) as ps2,
            tc.tile_pool(name="pacc", bufs=2) as pacc,
            tc.tile_pool(name="pst", bufs=4) as pst,
            tc.tile_pool(name="pgw", bufs=1) as pgw,
            tc.tile_pool(name="pb", bufs=1) as pb,
            tc.tile_pool(name="pp", bufs=8, space="PSUM") as pp,
        ):
            # -- PE warmup: dep-free matmuls; they run while the first
            # input DMAs land so the real MM stream starts at HAM 8/8.
            wut = pwu.tile([128, 512], bf16, tag="wu")
            nc.vector.memset(wut[:], 0)
            wups = pp.tile([128, 512], f32, tag="ps")
            for _ in range(WARMUP_MMS):
                nc.tensor.matmul(wups[:], wut[:, 0:128], wut[:],
                                 start=True, stop=True)

            # per-slot bias tiles
            b1s = [pb.tile([128, G * HJ], f32, tag=f"b1s{si}",
                           name=f"b1s{si}") for si in range(NSLOT)]
            b2s = [pb.tile([128, G * HJ], f32, tag=f"b2s{si}",
                           name=f"b2s{si}") for si in range(NSLOT)]
            bps = [pb.tile([128, KD], f32, tag=f"bps{si}", name=f"bps{si}")
                   for si in range(NSLOT)]

            # resident x^T, one tile per slot
            xblk = [px.tile([128, KD * sizes[si]], bf16, tag=f"x{si}",
                            name=f"x{si}") for si in range(NSLOT)]

            # gate weights broadcast [128, CAP]; first needed at window
            # (slot 0, g 7) ~ 1/4 into the kernel
            gwt = pgw.tile([128, CAP], f32, tag="gw")

            def new_w(si, g):
                w2h = [pw12.tile([128, 2048], bf16, tag=f"w2g{h}",
                                 name=f"w2g{si}_{g}_{h}") for h in range(2)]
                w1h = [pw12.tile([128, 2048], bf16, tag=f"w1g{h}",
                                 name=f"w1g{si}_{g}_{h}") for h in range(2)]
                wpg = pwp.tile([128, HJ * 1024], bf16, tag="wpg",
                               name=f"wpg{si}_{g}")
                return w2h, w1h, wpg

            def dma_w(si, g, w2h, w1h, wpg, early=False):
                # w2 before w1 (consumption order), halves split across
                # rings; wp halves split SWDGE + an alternating HWDGE ring
                if early:
                    w1engs = ((0, nc.gpsimd), (1, nc.sync))
                    wpengs = (nc.gpsimd, nc.scalar)
                else:
                    w1engs = ((0, nc.scalar), (1, nc.sync))
                    wpengs = (nc.gpsimd, nc.sync if g % 2 == 0 else nc.scalar)
                for half, eng in ((0, nc.sync), (1, nc.scalar)):
                    eng.dma_start(w2h[half][:],
                                  w2[si, g, :, half * 2048:(half + 1) * 2048])
                for half, eng in w1engs:
                    eng.dma_start(w1h[half][:],
                                  w1[si, g, :, half * 2048:(half + 1) * 2048])
                for half, eng in enumerate(wpengs):
                    eng.dma_start(wpg[:, half * 2048:(half + 1) * 2048],
                                  wp[si, g, :, half * 2048:(half + 1) * 2048])

            # late-input schedule: window index -> list of DMAs to emit
            # after that window's weight triggers (far deadlines only)
            def x_dma(si, h):
                o = KD * offs[si]
                m = KD * sizes[si] // 2
                return (xblk[si][:, h * m:(h + 1) * m],
                        xeT[:, o + h * m:o + h * m + m])

            late = {
                2: [(nc.sync, x_dma(1, 0)), (nc.scalar, x_dma(1, 1))],
                3: [(nc.gpsimd, (gwt[:], gwb[:]))],
                4: [(nc.sync, x_dma(2, 0)), (nc.scalar, x_dma(2, 1)),
                    (nc.gpsimd, (b2s[1][:], b2[1])),
                    (nc.gpsimd, (b1s[1][:], b1[1])),
                    (nc.gpsimd, (bps[1][:], bp[1]))],
                5: [(nc.sync, x_dma(3, 0)), (nc.scalar, x_dma(3, 1))],
                10: [(nc.gpsimd, (b2s[2][:], b2[2])),
                     (nc.gpsimd, (b1s[2][:], b1[2])),
                     (nc.gpsimd, (bps[2][:], bp[2]))],
                18: [(nc.gpsimd, (b2s[3][:], b2[3])),
                     (nc.gpsimd, (b1s[3][:], b1[3])),
                     (nc.gpsimd, (bps[3][:], bp[3]))],
            }

            def h_phase(si, g, w1h, w2h, fold_gw=False, chunks=None):
                sl = sizes[si]
                bo = offs[si]
                xt = xblk[si]
                if chunks is None:
                    # chunk-major x layout: chunk block at KD*c0, inner
                    # (k, t) with stride = chunk width
                    chunks = [(KD * c0, cw, c0, cw) for c0, cw in _chunks(sl)]
                hts = []
                for hj in range(HJ):
                    hm = g * HJ + hj
                    # hj-major weight layout: hj's block is 1024 cols
                    wco = (hj % 2) * 1024
                    w2t, w1t = w2h[hj // 2], w1h[hj // 2]
                    ht = pht.tile([128, sl], bf16, tag=f"h{hj}",
                                  name=f"h{si}_{g}_{hj}")
                    for ci, (xco, xs, c0, cw) in enumerate(chunks):
                        # ps2 first: silu overlaps the ps1 chain and both
                        # PSUM banks release sooner (w2 is DMA'd first)
                        ps2t = pp.tile([128, cw], f32, tag="ps",
                                       name=f"ps2_{si}_{g}_{hj}_{c0}")
                        for k in range(KD):
                            nc.tensor.matmul(
                                ps2t[:],
                                w2t[:, wco + k * 128:wco + k * 128 + 128],
                                xt[:, xco + k * xs:xco + k * xs + cw],
                                start=(k == 0), stop=(k == KD - 1))
                        s2 = ps2.tile([128, cw], f32, tag="s2",
                                      name=f"s2_{si}_{g}_{hj}_{c0}")
                        nc.scalar.activation(s2[:], ps2t[:], ACT.Silu,
                                             bias=b2s[si][:, hm:hm + 1])
                        if fold_gw:
                            # last slot: fold the gate into s2 so the
                            # epilogue is a single DVE add per dm
                            nc.vector.tensor_mul(s2[:], s2[:],
                                                 gwt[:, bo + c0:bo + c0 + cw])
                        ps1 = pp.tile([128, cw], f32, tag="ps",
                                      name=f"ps1_{si}_{g}_{hj}_{c0}")
                        for k in range(KD):
                            nc.tensor.matmul(
                                ps1[:],
                                w1t[:, wco + k * 128:wco + k * 128 + 128],
                                xt[:, xco + k * xs:xco + k * xs + cw],
                                start=(k == 0), stop=(k == KD - 1))
                        nc.vector.scalar_tensor_tensor(
                            ht[:, c0:c0 + cw], ps1[:], b1s[si][:, hm:hm + 1],
                            s2[:], op0=ALU.add, op1=ALU.mult)
                    hts.append(ht)
                return hts

            oacc = {}

            def dm_phase(si, g, wpg, hts, fold_gw=False, chunks=None):
                sl = sizes[si]
                bo = offs[si]
                if chunks is None:
                    chunks = _chunks(sl)
                if g == 0:
                    oacc[si] = [pacc.tile([128, sl], f32, tag=f"o{dm}",
                                          name=f"oacc{si}_{dm}")
                                for dm in range(KD)]
                if fold_gw:
                    # pre-scale the accumulator by the gate on the Pool
                    # engine (SBUF->SBUF), overlapping the psB chains, so
                    # the tail epilogue is a single DVE add per dm
                    for dm in range(KD):
                        nc.gpsimd.tensor_mul(oacc[si][dm][:],
                                             oacc[si][dm][:],
                                             gwt[:, bo:bo + sl])
                for dm in range(KD):
                    osl = oacc[si][dm]
                    for ci, (c0, cw) in enumerate(chunks):
                        psB = pp.tile([128, cw], f32, tag="ps",
                                      name=f"psB_{si}_{g}_{dm}_{c0}")
                        for hk in range(HJ):
                            # dm-major wp layout: dm's block is 512 cols
                            nc.tensor.matmul(
                                psB[:],
                                wpg[:, dm * 512 + hk * 128:dm * 512 + hk * 128 + 128],
                                hts[hk][:, c0:c0 + cw],
                                start=(hk == 0), stop=(hk == HJ - 1))
                        od = osl[:, c0:c0 + cw]
                        if g == 0:
                            # oacc = psB + bp; split between ACT and DVE
                            # so no engine paces the DMA-fed first group
                            # (GPSIMD cannot read PSUM)
                            if dm % 2 == 0:
                                nc.scalar.activation(od, psB[:], ACT.Identity,
                                                     bias=bps[si][:, dm:dm + 1])
                            else:
                                nc.vector.tensor_scalar_add(od, psB[:],
                                                            bps[si][:, dm:dm + 1])
                        elif g < G - 1:
                            nc.vector.tensor_add(od, od, psB[:])
                        elif fold_gw:
                            # gate already folded into oacc and ht
                            st = pst.tile([128, cw], bf16, tag="st",
                                          name=f"st{si}_{dm}_{c0}")
                            nc.vector.tensor_add(st[:], od, psB[:])
                            eng = (nc.sync, nc.scalar, nc.gpsimd)[dm % 3]
                            eng.dma_start(
                                outT[dm * 128:(dm + 1) * 128,
                                     bo + c0:bo + c0 + cw], st[:])
                        else:
                            # epilogue: out = (oacc + psB) * gate, streamed
                            # out (bf16) per (slot, dm); the SBUF->SBUF
                            # gate multiply goes to the Pool engine
                            sa = pst.tile([128, cw], f32, tag="sa",
                                          name=f"sa{si}_{dm}_{c0}")
                            nc.vector.tensor_add(sa[:], od, psB[:])
                            st = pst.tile([128, cw], bf16, tag="st",
                                          name=f"st{si}_{dm}_{c0}")
                            nc.gpsimd.tensor_mul(st[:], sa[:],
                                                 gwt[:, bo + c0:bo + c0 + cw])
                            eng = (nc.sync, nc.scalar, nc.gpsimd)[dm % 3]
                            eng.dma_start(
                                outT[dm * 128:(dm + 1) * 128,
                                     bo + c0:bo + c0 + cw], st[:])

            # ---- main: slot-major, windows (slot, g), software-pipelined
            windows = [(si, g) for si in range(NSLOT) for g in range(G)]
            prev = None  # (si, g, wpg, hts) of the previous window
            for wi, (si, g) in enumerate(windows):
                w2h, w1h, wpg = new_w(si, g)
                if wi == 0:
                    # deadline-ordered prologue: the first h-chain needs
                    # x slot 0 (all k) + w2's hj0 block; stream the rest
                    # in consumption order (w2/w1 alternate per hj)
                    # across all three rings.
                    sl0 = sizes[0]

                    def xpc(i):  # x eighth, 1 k-tile (128KB)
                        return (xblk[0][:, i * sl0:(i + 1) * sl0],
                                xeT[:, i * sl0:(i + 1) * sl0])

                    def wb(wt, ws, hj):  # hj block, 1024 cols (256KB)
                        return (wt[hj // 2][:, (hj % 2) * 1024:(hj % 2) * 1024 + 1024],
                                ws[0, 0, :, hj * 1024:(hj + 1) * 1024])

                    for dst, src in (xpc(0), xpc(3), xpc(6),
                                     (b2s[0][:], b2[0]), (b1s[0][:], b1[0]),
                                     wb(w2h, w2, 1), wb(w1h, w1, 2)):
                        nc.sync.dma_start(dst, src)
                    for dst, src in (xpc(1), xpc(4), wb(w1h, w1, 0),
                                     xpc(7), wb(w2h, w2, 3),
                                     (bps[0][:], bp[0]),
                                     (wpg[:, 2048:4096], wp[0, 0, :, 2048:4096])):
                        nc.scalar.dma_start(dst, src)
                    for dst, src in (xpc(2), xpc(5), wb(w2h, w2, 0),
                                     wb(w1h, w1, 1), wb(w2h, w2, 2),
                                     wb(w1h, w1, 3),
                                     (wpg[:, 0:2048], wp[0, 0, :, 0:2048])):
                        nc.gpsimd.dma_start(dst, src)
                else:
                    dma_w(si, g, w2h, w1h, wpg, early=(wi <= 3))
                for eng, (dst, src) in late.get(wi, ()):
                    eng.dma_start(dst, src)

                last = wi == len(windows) - 1
                if last and prev is not None:
                    # final window un-pipelined: drain the previous
                    # window's dm-phase (and its DVE adds / PSUM banks)
                    # before the last h-phase, so the kernel tail is just
                    # the last dm-phase + epilogue
                    dm_phase(*prev)
                    prev = None
                hts = h_phase(si, g, w1h, w2h, fold_gw=last)
                if prev is not None:
                    dm_phase(*prev)
                prev = (si, g, wpg, hts)
            dm_phase(prev[0], prev[1], prev[2], prev[3], fold_gw=True)

    nc.finalize()
    return nc


def _route(x2d, noise2d, gate_w, noise_weight, kk):
    T = x2d.shape[0]
    logits = x2d @ gate_w
    logits = logits + noise2d * noise_weight[None, :]
    kk = int(kk)
    Ee = logits.shape[1]
    if kk >= Ee:
        sel = np.ones((T, Ee), dtype=bool)
    else:
        part = np.argpartition(-logits, kk - 1, axis=1)[:, :kk]
        sel = np.zeros((T, Ee), dtype=bool)
        sel[np.arange(T)[:, None], part] = True
    mx = logits.max(axis=1, keepdims=True)
    ex = np.exp(logits - mx, dtype=np.float32) * sel
    gw = ex / ex.sum(axis=1, keepdims=True)
    return sel, gw.astype(np.float32)


def _pack(counts):
    """Assign each expert a multiset of slot position-types (8 instances
    of each of the 4 per-core slot sizes) covering its token count.

    Returns (sizes, alloc) with alloc[e] = per-position instance counts.
    DP over experts, state = per-position instances used, min total slack.
    """
    import itertools
    # position 0 feeds the prologue and the last position feeds the tail
    # epilogue; candidates ordered by capacity (prefer minimal padding)
    cands = [
        (512, 544, 504, 496),   # cap 2056
        (512, 544, 512, 496),   # cap 2064
        (512, 544, 512, 512),   # cap 2080
        (512, 576, 512, 512),
        (512, 608, 512, 512),
        (512, 640, 512, 512),
        (512, 768, 512, 512),
        (512, 1024, 512, 512),
        (768, 1024, 768, 768),
    ]
    for sizes in cands:
        maxs = max(sizes)
        combos = []
        for e in range(E):
            n = counts[e]
            ce = []
            for a in itertools.product(range(7), repeat=NSLOT):
                if sum(a) > 6:
                    continue
                cap = sum(x * s for x, s in zip(a, sizes))
                if cap >= n and cap - maxs < n:
                    ce.append((a, cap - n))
            combos.append(ce)
        dp = {(0,) * NSLOT: (0, [])}
        for e in range(E):
            nd = {}
            for st, (sl, hist) in dp.items():
                for a, s in combos[e]:
                    k2 = tuple(u + x for u, x in zip(st, a))
                    if max(k2) > 8:
                        continue
                    v = sl + s
                    if k2 not in nd or v < nd[k2][0]:
                        nd[k2] = (v, hist + [a])
            dp = nd
            if not dp:
                break
        if dp and (8,) * NSLOT in dp:
            return sizes, dp[(8,) * NSLOT][1]
    raise RuntimeError(f"no slot packing found for counts {counts}")


def _plan(counts):
    """Build the per-core slot plan.

    Returns (sizes, plan) with plan[core] = list over slot positions of
    (expert, n_tokens_in_this_slot).
    """
    sizes, alloc = _pack(list(counts))
    # per position-type, the expert owning each of its 8 instances
    inst = [[] for _ in range(NSLOT)]
    for e, a in enumerate(alloc):
        for pos in range(NSLOT):
            inst[pos] += [e] * a[pos]
    assert all(len(i) == 8 for i in inst)
    remaining = list(counts)
    plan = []
    for core in range(8):
        slots = []
        for pos, sz in enumerate(sizes):
            e = inst[pos][core]
            take = min(remaining[e], sz)
            remaining[e] -= take
            slots.append((e, take))
        plan.append(slots)
    assert all(r == 0 for r in remaining), (remaining, alloc)
    return sizes, plan


def _prep_maps(x2d, gw, idxs, sizes, plan, w1, b1, w2, b2, wp, bp):
    import ml_dtypes
    bf16 = ml_dtypes.bfloat16
    CAP = sum(sizes)
    # per-expert weight prep (done once, referenced per slot):
    # w1/w2 -> [G, 128, (hj, k, 128)], wp -> [G, 128, (dm, hk, 128)]
    w1p, w2p, wpp, b1p, b2p, bpp = [], [], [], [], [], []
    for e in range(E):
        w1p.append(w1[e].reshape(KD, 128, G, HJ, 128)
                   .transpose(2, 1, 3, 0, 4)
                   .reshape(G, 128, HJ * KD * 128).astype(bf16))
        w2p.append(w2[e].reshape(KD, 128, G, HJ, 128)
                   .transpose(2, 1, 3, 0, 4)
                   .reshape(G, 128, HJ * KD * 128).astype(bf16))
        wpp.append(wp[e].reshape(G, HJ, 128, KD, 128)
                   .transpose(0, 2, 3, 1, 4)
                   .reshape(G, 128, KD * HJ * 128).astype(bf16))
        b1p.append(np.ascontiguousarray(
            b1[e].reshape(G * HJ, 128).T.astype(np.float32)))
        b2p.append(np.ascontiguousarray(
            b2[e].reshape(G * HJ, 128).T.astype(np.float32)))
        bpp.append(np.ascontiguousarray(
            bp[e].reshape(KD, 128).T.astype(np.float32)))

    used = [0] * E
    in_maps = []
    core_tok = []
    for core in range(8):
        xeT = np.zeros((128, KD * CAP), dtype=bf16)
        gwb = np.zeros((128, CAP), dtype=np.float32)
        w1in = np.empty((NSLOT, G, 128, HJ * KD * 128), dtype=bf16)
        w2in = np.empty((NSLOT, G, 128, HJ * KD * 128), dtype=bf16)
        wpin = np.empty((NSLOT, G, 128, KD * HJ * 128), dtype=bf16)
        b1in = np.empty((NSLOT, 128, G * HJ), dtype=np.float32)
        b2in = np.empty((NSLOT, 128, G * HJ), dtype=np.float32)
        bpin = np.empty((NSLOT, 128, KD), dtype=np.float32)
        toks = []
        off = 0
        for pos, (e, take) in enumerate(plan[core]):
            sz = sizes[pos]
            idx = idxs[e][used[e]:used[e] + take]
            used[e] += take
            toks.append(idx)
            xs = np.zeros((KD, 128, sz), dtype=bf16)
            xs[:, :, :take] = (x2d[idx].T.astype(bf16)
                               .reshape(KD, 128, take))
            # chunk-major within the slot: [(chunk), k, t]
            xeT[:, KD * off:KD * (off + sz)] = np.concatenate(
                [xs[:, :, c0:c0 + cwd].transpose(1, 0, 2).reshape(128, KD * cwd)
                 for (c0, cwd) in _chunks(sz)], axis=1)
            gwb[:, off:off + take] = gw[idx, e][None, :]
            w1in[pos] = w1p[e]
            w2in[pos] = w2p[e]
            wpin[pos] = wpp[e]
            b1in[pos] = b1p[e]
            b2in[pos] = b2p[e]
            bpin[pos] = bpp[e]
            off += sz
        core_tok.append(toks)
        in_maps.append({
            "xeT": xeT, "w1": w1in, "w2": w2in, "wp": wpin,
            "b1": b1in, "b2": b2in, "bp": bpin, "gwb": gwb,
        })
    return in_maps, core_tok


def kernel(**inputs):
    from concourse.bass_utils import run_bass_kernel_spmd

    x = np.asarray(inputs["x"], dtype=np.float32)
    noise = np.asarray(inputs["noise"], dtype=np.float32)
    gate_w = np.asarray(inputs["gate_w"], dtype=np.float32)
    noise_weight = np.asarray(inputs["noise_weight"], dtype=np.float32)
    w1 = np.asarray(inputs["w1"], dtype=np.float32)
    b1 = np.asarray(inputs["b1"], dtype=np.float32)
    w2 = np.asarray(inputs["w2"], dtype=np.float32)
    b2 = np.asarray(inputs["b2"], dtype=np.float32)
    wp = np.asarray(inputs["wp"], dtype=np.float32)
    bp = np.asarray(inputs["bp"], dtype=np.float32)
    kk = int(np.asarray(inputs["k"]))

    B, S, _ = x.shape
    T = B * S
    x2d = np.ascontiguousarray(x.reshape(T, D))
    noise2d = noise.reshape(T, E)

    sel, gw = _route(x2d, noise2d, gate_w, noise_weight, kk)
    idxs = [np.nonzero(sel[:, e])[0] for e in range(E)]
    counts = [len(i) for i in idxs]
    sizes, plan = _plan(counts)

    if sizes not in _NC_CACHE:
        _NC_CACHE[sizes] = _build(sizes)
    nc = _NC_CACHE[sizes]

    in_maps, core_tok = _prep_maps(x2d, gw, idxs, sizes, plan,
                                   w1, b1, w2, b2, wp, bp)
    res = run_bass_kernel_spmd(nc, in_maps, core_ids=list(range(8))).results

    y2d = np.zeros((T, D), dtype=np.float32)
    for core in range(8):
        off = 0
        for pos, idx in enumerate(core_tok[core]):
            n = len(idx)
            if n:
                y2d[idx] += (res[core]["outT"][:, off:off + n]
                             .astype(np.float32).T)
            off += sizes[pos]
    return y2d.reshape(B, S, D)


# revision 32
# speedup vs baseline: 1.0001x; 1.0001x over previous
"""Expert-parallel MoE (top-k routing + SwiGLU experts) for 8 Trainium2 cores.

Strategy (balanced slot-packing, slot-major, deadline-ordered prologue)
-----------------------------------------------------------------------
- Host computes the (tiny) gate: logits = x @ gate_w (+ noise * noise_weight),
  top-k selection, sparse softmax weights.  0.03% of total FLOPs.
- Load balancing: instead of one expert per core padded to the max expert's
  token count (C=2176 for these counts), every core gets 4 weight-SLOTS of
  sizes (512, 544, 504, 496) -- capacity 2056 vs. the perfect 2048.  A slot
  holds tokens of a single expert; a small DP assigns each expert a multiset
  of slot instances across cores so all 16384 (token, expert) pairs fit with
  minimal padding.  Each slot's weights are streamed independently (the
  uniform SPMD program cannot dedup same-expert slots), ~96 MB/core of HBM
  reads -- fine: the 3 DMA queues burst at 150-190 GB/s each and sit mostly
  idle.
- Slot-major loop: for slot s: for h-group g: stream w1/w2/wp(s, g),
  accumulate out_acc(s) over g; at g==7 the epilogue streams the slot's
  output DMA immediately, so the kernel tail is just the last slot's
  last-dm epilogue + drain.  The final window is un-pipelined and uses the
  fold-gw epilogue (gate folded into ht and pre-scaled into oacc on the
  Pool engine) so the tail after the last matmul is one DVE add per dm.
- w1/w2 SBUF layout is hj-major ([128, (hj, k, 128)]) and wp is dm-major
  ([128, (dm, hk, 128)]); x is chunk-major per slot.  The first h-chain
  then only needs x-slot0 + one 256KB weight block; the prologue streams
  ~6MB in consumption-deadline order across all three DMA rings while the
  PE computes behind it.
- Device kernel math (tokens on the free axis; bf16 matmul inputs, f32
  PSUM accumulation):
    hT[128h, tok] = (w1g.T @ xT + b1) * silu(w2g.T @ xT + b2)   (bf16)
    out_acc[128d, tok] += wpg.T @ hT          (PSUM acc over the 512 h)
  The 544-slot runs as 2x272-wide matmuls (PSUM bank is 512 f32 wide;
  >=128-wide moving keeps the stationary-weight load hidden).
- Software pipelining across (slot, g) windows: window w's dm-phase (psB
  chains) is emitted after window w+1's h-phase so the PE FIFO never waits
  on the cross-engine silu/STT chain.  Mid-kernel epilogue gate-multiplies
  go to the Pool engine (GPSIMD cannot read PSUM; adds stay on DVE).

Measured (core 0 NTFF): 695.6us vs 746.1us for the one-expert-per-core
baseline; PE busy 674us at ~96% of bf16 peak, HBM reads ~106MB/core.
"""

import sys
import numpy as np

sys.path.insert(0, "/opt/trn_rl_repo")

D = 1024
H = 4096
E = 8
KD = D // 128          # 8 k-tiles over D
G = 8                  # h-groups
HJ = 4                 # 128-row h-tiles per group (G*HJ*128 == H)
NSLOT = 4
WARMUP_MMS = 22   # dep-free PE warmup bridging the prologue DMA fill so
                  # the HAM clock never drops to 4/8 before the first
                  # real chains (~13us in)

_NC_CACHE = {}


def _chunks(sl):
    """Split a slot of sl tokens into matmul-width chunks (<=512, >=128)."""
    if sl <= 512:
        return [(0, sl)]
    half = (sl + 1) // 2
    half = ((half + 15) // 16) * 16
    return [(0, half), (half, sl - half)]


def _build(sizes):
    import concourse.mybir as mybir
    import concourse.tile as tile
    from concourse import bacc

    f32 = mybir.dt.float32
    bf16 = mybir.dt.bfloat16
    ACT = mybir.ActivationFunctionType
    ALU = mybir.AluOpType

    CAP = sum(sizes)
    offs = [sum(sizes[:i]) for i in range(NSLOT)]

    nc = bacc.Bacc()
    # all inputs pre-arranged on the host into SBUF tile layout
    xeT = nc.dram_tensor("xeT", [128, KD * CAP], bf16, kind="ExternalInput")
    w1 = nc.dram_tensor("w1", [NSLOT, G, 128, HJ * KD * 128], bf16,
                        kind="ExternalInput")
    w2 = nc.dram_tensor("w2", [NSLOT, G, 128, HJ * KD * 128], bf16,
                        kind="ExternalInput")
    wp = nc.dram_tensor("wp", [NSLOT, G, 128, KD * HJ * 128], bf16,
                        kind="ExternalInput")
    b1 = nc.dram_tensor("b1", [NSLOT, 128, G * HJ], f32, kind="ExternalInput")
    b2 = nc.dram_tensor("b2", [NSLOT, 128, G * HJ], f32, kind="ExternalInput")
    bp = nc.dram_tensor("bp", [NSLOT, 128, KD], f32, kind="ExternalInput")
    gwb = nc.dram_tensor("gwb", [128, CAP], f32, kind="ExternalInput")
    outT = nc.dram_tensor("outT", [D, CAP], bf16, kind="ExternalOutput")

    with tile.TileContext(nc) as tc:
        with (
            tc.tile_pool(name="pwu", bufs=1) as pwu,
            tc.tile_pool(name="pw12", bufs=3) as pw12,
            tc.tile_pool(name="pwp", bufs=3) as pwp,
            tc.tile_pool(name="px", bufs=1) as px,
            tc.tile_pool(name="pht", bufs=2) as pht,
            tc.tile_pool(name="ps2", bufs=3) as ps2,
            tc.tile_pool(name="pacc", bufs=2) as pacc,
            tc.tile_pool(name="pst", bufs=4) as pst,
            tc.tile_pool(name="pgw", bufs=1) as pgw,
            tc.tile_pool(name="pb", bufs=1) as pb,
            tc.tile_pool(name="pp", bufs=8, space="PSUM") as pp,
        ):
            # -- PE warmup: dep-free matmuls; they run while the first
            # input DMAs land so the real MM stream starts at HAM 8/8.
            wut = pwu.tile([128, 512], bf16, tag="wu")
            nc.vector.memset(wut[:], 0)
            wups = pp.tile([128, 512], f32, tag="ps")
            for _ in range(WARMUP_MMS):
                nc.tensor.matmul(wups[:], wut[:, 0:128], wut[:],
                                 start=True, stop=True)

            # per-slot bias tiles
            b1s = [pb.tile([128, G * HJ], f32, tag=f"b1s{si}",
                           name=f"b1s{si}") for si in range(NSLOT)]
            b2s = [pb.tile([128, G * HJ], f32, tag=f"b2s{si}",
                           name=f"b2s{si}") for si in range(NSLOT)]
            bps = [pb.tile([128, KD], f32, tag=f"bps{si}", name=f"bps{si}")
                   for si in range(NSLOT)]

            # resident x^T, one tile per slot
            xblk = [px.tile([128, KD * sizes[si]], bf16, tag=f"x{si}",
                            name=f"x{si}") for si in range(NSLOT)]

            # gate weights broadcast [128, CAP]; first needed at window
            # (slot 0, g 7) ~ 1/4 into the kernel
            gwt = pgw.tile([128, CAP], f32, tag="gw")

            def new_w(si, g):
                w2h = [pw12.tile([128, 2048], bf16, tag=f"w2g{h}",
                                 name=f"w2g{si}_{g}_{h}") for h in range(2)]
                w1h = [pw12.tile([128, 2048], bf16, tag=f"w1g{h}",
                                 name=f"w1g{si}_{g}_{h}") for h in range(2)]
                wpg = pwp.tile([128, HJ * 1024], bf16, tag="wpg",
                               name=f"wpg{si}_{g}")
                return w2h, w1h, wpg

            def dma_w(si, g, w2h, w1h, wpg, early=False):
                # w2 before w1 (consumption order), halves split across
                # rings; wp halves split SWDGE + an alternating HWDGE ring
                if early:
                    w1engs = ((0, nc.gpsimd), (1, nc.sync))
                    wpengs = (nc.gpsimd, nc.scalar)
                else:
                    w1engs = ((0, nc.scalar), (1, nc.sync))
                    wpengs = (nc.gpsimd, nc.sync if g % 2 == 0 else nc.scalar)
                for half, eng in ((0, nc.sync), (1, nc.scalar)):
                    eng.dma_start(w2h[half][:],
                                  w2[si, g, :, half * 2048:(half + 1) * 2048])
                for half, eng in w1engs:
                    eng.dma_start(w1h[half][:],
                                  w1[si, g, :, half * 2048:(half + 1) * 2048])
                for half, eng in enumerate(wpengs):
                    eng.dma_start(wpg[:, half * 2048:(half + 1) * 2048],
                                  wp[si, g, :, half * 2048:(half + 1) * 2048])

            # late-input schedule: window index -> list of DMAs to emit
            # after that window's weight triggers (far deadlines only)
            def x_dma(si, h):
                o = KD * offs[si]
                m = KD * sizes[si] // 2
                return (xblk[si][:, h * m:(h + 1) * m],
                        xeT[:, o + h * m:o + h * m + m])

            late = {
                3: [(nc.gpsimd, (gwt[:], gwb[:]))],
                4: [(nc.sync, x_dma(1, 0)), (nc.scalar, x_dma(1, 1)),
                    (nc.gpsimd, (b2s[1][:], b2[1])),
                    (nc.gpsimd, (b1s[1][:], b1[1])),
                    (nc.gpsimd, (bps[1][:], bp[1]))],
                6: [(nc.sync, x_dma(2, 0)), (nc.scalar, x_dma(2, 1))],
                10: [(nc.sync, x_dma(3, 0)), (nc.scalar, x_dma(3, 1)),
                     (nc.gpsimd, (b2s[2][:], b2[2])),
                     (nc.gpsimd, (b1s[2][:], b1[2])),
                     (nc.gpsimd, (bps[2][:], bp[2]))],
                18: [(nc.gpsimd, (b2s[3][:], b2[3])),
                     (nc.gpsimd, (b1s[3][:], b1[3])),
                     (nc.gpsimd, (bps[3][:], bp[3]))],
            }

            def h_phase(si, g, w1h, w2h, fold_gw=False, chunks=None):
                sl = sizes[si]
                bo = offs[si]
                xt = xblk[si]
                if chunks is None:
                    # chunk-major x layout: chunk block at KD*c0, inner
                    # (k, t) with stride = chunk width
                    chunks = [(KD * c0, cw, c0, cw) for c0, cw in _chunks(sl)]
                hts = []
                for hj in range(HJ):
                    hm = g * HJ + hj
                    # hj-major weight layout: hj's block is 1024 cols
                    wco = (hj % 2) * 1024
                    w2t, w1t = w2h[hj // 2], w1h[hj // 2]
                    ht = pht.tile([128, sl], bf16, tag=f"h{hj}",
                                  name=f"h{si}_{g}_{hj}")
                    for ci, (xco, xs, c0, cw) in enumerate(chunks):
                        # ps2 first: silu overlaps the ps1 chain and both
                        # PSUM banks release sooner (w2 is DMA'd first)
                        ps2t = pp.tile([128, cw], f32, tag="ps",
                                       name=f"ps2_{si}_{g}_{hj}_{c0}")
                        for k in range(KD):
                            nc.tensor.matmul(
                                ps2t[:],
                                w2t[:, wco + k * 128:wco + k * 128 + 128],
                                xt[:, xco + k * xs:xco + k * xs + cw],
                                start=(k == 0), stop=(k == KD - 1))
                        s2 = ps2.tile([128, cw], f32, tag="s2",
                                      name=f"s2_{si}_{g}_{hj}_{c0}")
                        nc.scalar.activation(s2[:], ps2t[:], ACT.Silu,
                                             bias=b2s[si][:, hm:hm + 1])
                        if fold_gw:
                            # last slot: fold the gate into s2 so the
                            # epilogue is a single DVE add per dm
                            nc.vector.tensor_mul(s2[:], s2[:],
                                                 gwt[:, bo + c0:bo + c0 + cw])
                        ps1 = pp.tile([128, cw], f32, tag="ps",
                                      name=f"ps1_{si}_{g}_{hj}_{c0}")
                        for k in range(KD):
                            nc.tensor.matmul(
                                ps1[:],
                                w1t[:, wco + k * 128:wco + k * 128 + 128],
                                xt[:, xco + k * xs:xco + k * xs + cw],
                                start=(k == 0), stop=(k == KD - 1))
                        nc.vector.scalar_tensor_tensor(
                            ht[:, c0:c0 + cw], ps1[:], b1s[si][:, hm:hm + 1],
                            s2[:], op0=ALU.add, op1=ALU.mult)
                    hts.append(ht)
                return hts

            oacc = {}

            def dm_phase(si, g, wpg, hts, fold_gw=False, chunks=None):
                sl = sizes[si]
                bo = offs[si]
                if chunks is None:
                    chunks = _chunks(sl)
                if g == 0:
                    oacc[si] = [pacc.tile([128, sl], f32, tag=f"o{dm}",
                                          name=f"oacc{si}_{dm}")
                                for dm in range(KD)]
                if fold_gw:
                    # pre-scale the accumulator by the gate on the Pool
                    # engine (SBUF->SBUF), overlapping the psB chains, so
                    # the tail epilogue is a single DVE add per dm
                    for dm in range(KD):
                        nc.gpsimd.tensor_mul(oacc[si][dm][:],
                                             oacc[si][dm][:],
                                             gwt[:, bo:bo + sl])
                for dm in range(KD):
                    osl = oacc[si][dm]
                    for ci, (c0, cw) in enumerate(chunks):
                        psB = pp.tile([128, cw], f32, tag="ps",
                                      name=f"psB_{si}_{g}_{dm}_{c0}")
                        for hk in range(HJ):
                            # dm-major wp layout: dm's block is 512 cols
                            nc.tensor.matmul(
                                psB[:],
                                wpg[:, dm * 512 + hk * 128:dm * 512 + hk * 128 + 128],
                                hts[hk][:, c0:c0 + cw],
                                start=(hk == 0), stop=(hk == HJ - 1))
                        od = osl[:, c0:c0 + cw]
                        if g == 0:
                            # oacc = psB + bp; split between ACT and DVE
                            # so no engine paces the DMA-fed first group
                            # (GPSIMD cannot read PSUM)
                            if dm % 2 == 0:
                                nc.scalar.activation(od, psB[:], ACT.Identity,
                                                     bias=bps[si][:, dm:dm + 1])
                            else:
                                nc.vector.tensor_scalar_add(od, psB[:],
                                                            bps[si][:, dm:dm + 1])
                        elif g < G - 1:
                            nc.vector.tensor_add(od, od, psB[:])
                        elif fold_gw:
                            # gate already folded into oacc and ht
                            st = pst.tile([128, cw], bf16, tag="st",
                                          name=f"st{si}_{dm}_{c0}")
                            nc.vector.tensor_add(st[:], od, psB[:])
                            eng = (nc.sync, nc.scalar, nc.gpsimd)[dm % 3]
                            eng.dma_start(
                                outT[dm * 128:(dm + 1) * 128,
                                     bo + c0:bo + c0 + cw], st[:])
                        else:
                            # epilogue: out = (oacc + psB) * gate, streamed
                            # out (bf16) per (slot, dm); the SBUF->SBUF
                            # gate multiply goes to the Pool engine
                            sa = pst.tile([128, cw], f32, tag="sa",
                                          name=f"sa{si}_{dm}_{c0}")
                            nc.vector.tensor_add(sa[:], od, psB[:])
                            st = pst.tile([128, cw], bf16, tag="st",
                                          name=f"st{si}_{dm}_{c0}")
                            nc.gpsimd.tensor_mul(st[:], sa[:],
                                                 gwt[:, bo + c0:bo + c0 + cw])
                            eng = (nc.sync, nc.scalar, nc.gpsimd)[dm % 3]
                            eng.dma_start(
                                outT[dm * 128:(dm + 1) * 128,
                                     bo + c0:bo + c0 + cw], st[:])

            # ---- main: slot-major, windows (slot, g), software-pipelined
            windows = [(si, g) for si in range(NSLOT) for g in range(G)]
            prev = None  # (si, g, wpg, hts) of the previous window
            for wi, (si, g) in enumerate(windows):
                w2h, w1h, wpg = new_w(si, g)
                if wi == 0:
                    # deadline-ordered prologue: the first h-chain needs
                    # x slot 0 (all k) + w2's hj0 block; stream the rest
                    # in consumption order (w2/w1 alternate per hj)
                    # across all three rings.
                    sl0 = sizes[0]

                    def xpc(i):  # x eighth, 1 k-tile (128KB)
                        return (xblk[0][:, i * sl0:(i + 1) * sl0],
                                xeT[:, i * sl0:(i + 1) * sl0])

                    def wb(wt, ws, hj):  # hj block, 1024 cols (256KB)
                        return (wt[hj // 2][:, (hj % 2) * 1024:(hj % 2) * 1024 + 1024],
                                ws[0, 0, :, hj * 1024:(hj + 1) * 1024])

                    for dst, src in (xpc(0), xpc(3), xpc(6),
                                     (b2s[0][:], b2[0]), (b1s[0][:], b1[0]),
                                     wb(w2h, w2, 1), wb(w1h, w1, 2)):
                        nc.sync.dma_start(dst, src)
                    for dst, src in (xpc(1), xpc(4), wb(w1h, w1, 0),
                                     xpc(7), wb(w2h, w2, 3),
                                     (bps[0][:], bp[0]),
                                     (wpg[:, 2048:4096], wp[0, 0, :, 2048:4096])):
                        nc.scalar.dma_start(dst, src)
                    for dst, src in (xpc(2), xpc(5), wb(w2h, w2, 0),
                                     wb(w1h, w1, 1), wb(w2h, w2, 2),
                                     wb(w1h, w1, 3),
                                     (wpg[:, 0:2048], wp[0, 0, :, 0:2048])):
                        nc.gpsimd.dma_start(dst, src)
                else:
                    dma_w(si, g, w2h, w1h, wpg, early=(wi <= 3))
                for eng, (dst, src) in late.get(wi, ()):
                    eng.dma_start(dst, src)

                last = wi == len(windows) - 1
                if last and prev is not None:
                    # final window un-pipelined: drain the previous
                    # window's dm-phase (and its DVE adds / PSUM banks)
                    # before the last h-phase, so the kernel tail is just
                    # the last dm-phase + epilogue
                    dm_phase(*prev)
                    prev = None
                hts = h_phase(si, g, w1h, w2h, fold_gw=last)
                if prev is not None:
                    dm_phase(*prev)
                prev = (si, g, wpg, hts)
            dm_phase(prev[0], prev[1], prev[2], prev[3], fold_gw=True)

    nc.finalize()
    return nc


def _route(x2d, noise2d, gate_w, noise_weight, kk):
    T = x2d.shape[0]
    logits = x2d @ gate_w
    logits = logits + noise2d * noise_weight[None, :]
    kk = int(kk)
    Ee = logits.shape[1]
    if kk >= Ee:
        sel = np.ones((T, Ee), dtype=bool)
    else:
        part = np.argpartition(-logits, kk - 1, axis=1)[:, :kk]
        sel = np.zeros((T, Ee), dtype=bool)
        sel[np.arange(T)[:, None], part] = True
    mx = logits.max(axis=1, keepdims=True)
    ex = np.exp(logits - mx, dtype=np.float32) * sel
    gw = ex / ex.sum(axis=1, keepdims=True)
    return sel, gw.astype(np.float32)


def _pack(counts):
    """Assign each expert a multiset of slot position-types (8 instances
    of each of the 4 per-core slot sizes) covering its token count.

    Returns (sizes, alloc) with alloc[e] = per-position instance counts.
    DP over experts, state = per-position instances used, min total slack.
    """
    import itertools
    # position 0 feeds the prologue and the last position feeds the tail
    # epilogue; candidates ordered by capacity (prefer minimal padding)
    cands = [
        (512, 544, 504, 496),   # cap 2056
        (512, 544, 512, 496),   # cap 2064
        (512, 544, 512, 512),   # cap 2080
        (512, 576, 512, 512),
        (512, 608, 512, 512),
        (512, 640, 512, 512),
        (512, 768, 512, 512),
        (512, 1024, 512, 512),
        (768, 1024, 768, 768),
    ]
    for sizes in cands:
        maxs = max(sizes)
        combos = []
        for e in range(E):
            n = counts[e]
            ce = []
            for a in itertools.product(range(7), repeat=NSLOT):
                if sum(a) > 6:
                    continue
                cap = sum(x * s for x, s in zip(a, sizes))
                if cap >= n and cap - maxs < n:
                    ce.append((a, cap - n))
            combos.append(ce)
        dp = {(0,) * NSLOT: (0, [])}
        for e in range(E):
            nd = {}
            for st, (sl, hist) in dp.items():
                for a, s in combos[e]:
                    k2 = tuple(u + x for u, x in zip(st, a))
                    if max(k2) > 8:
                        continue
                    v = sl + s
                    if k2 not in nd or v < nd[k2][0]:
                        nd[k2] = (v, hist + [a])
            dp = nd
            if not dp:
                break
        if dp and (8,) * NSLOT in dp:
            return sizes, dp[(8,) * NSLOT][1]
    raise RuntimeError(f"no slot packing found for counts {counts}")


def _plan(counts):
    """Build the per-core slot plan.

    Returns (sizes, plan) with plan[core] = list over slot positions of
    (expert, n_tokens_in_this_slot).
    """
    sizes, alloc = _pack(list(counts))
    # per position-type, the expert owning each of its 8 instances
    inst = [[] for _ in range(NSLOT)]
    for e, a in enumerate(alloc):
        for pos in range(NSLOT):
            inst[pos] += [e] * a[pos]
    assert all(len(i) == 8 for i in inst)
    remaining = list(counts)
    plan = []
    for core in range(8):
        slots = []
        for pos, sz in enumerate(sizes):
            e = inst[pos][core]
            take = min(remaining[e], sz)
            remaining[e] -= take
            slots.append((e, take))
        plan.append(slots)
    assert all(r == 0 for r in remaining), (remaining, alloc)
    return sizes, plan


def _prep_maps(x2d, gw, idxs, sizes, plan, w1, b1, w2, b2, wp, bp):
    import ml_dtypes
    bf16 = ml_dtypes.bfloat16
    CAP = sum(sizes)
    # per-expert weight prep (done once, referenced per slot):
    # w1/w2 -> [G, 128, (hj, k, 128)], wp -> [G, 128, (dm, hk, 128)]
    w1p, w2p, wpp, b1p, b2p, bpp = [], [], [], [], [], []
    for e in range(E):
        w1p.append(w1[e].reshape(KD, 128, G, HJ, 128)
                   .transpose(2, 1, 3, 0, 4)
                   .reshape(G, 128, HJ * KD * 128).astype(bf16))
        w2p.append(w2[e].reshape(KD, 128, G, HJ, 128)
                   .transpose(2, 1, 3, 0, 4)
                   .reshape(G, 128, HJ * KD * 128).astype(bf16))
        wpp.append(wp[e].reshape(G, HJ, 128, KD, 128)
                   .transpose(0, 2, 3, 1, 4)
                   .reshape(G, 128, KD * HJ * 128).astype(bf16))
        b1p.append(np.ascontiguousarray(
            b1[e].reshape(G * HJ, 128).T.astype(np.float32)))
        b2p.append(np.ascontiguousarray(
            b2[e].reshape(G * HJ, 128).T.astype(np.float32)))
        bpp.append(np.ascontiguousarray(
            bp[e].reshape(KD, 128).T.astype(np.float32)))

    used = [0] * E
    in_maps = []
    core_tok = []
    for core in range(8):
        xeT = np.zeros((128, KD * CAP), dtype=bf16)
        gwb = np.zeros((128, CAP), dtype=np.float32)
        w1in = np.empty((NSLOT, G, 128, HJ * KD * 128), dtype=bf16)
        w2in = np.empty((NSLOT, G, 128, HJ * KD * 128), dtype=bf16)
        wpin = np.empty((NSLOT, G, 128, KD * HJ * 128), dtype=bf16)
        b1in = np.empty((NSLOT, 128, G * HJ), dtype=np.float32)
        b2in = np.empty((NSLOT, 128, G * HJ), dtype=np.float32)
        bpin = np.empty((NSLOT, 128, KD), dtype=np.float32)
        toks = []
        off = 0
        for pos, (e, take) in enumerate(plan[core]):
            sz = sizes[pos]
            idx = idxs[e][used[e]:used[e] + take]
            used[e] += take
            toks.append(idx)
            xs = np.zeros((KD, 128, sz), dtype=bf16)
            xs[:, :, :take] = (x2d[idx].T.astype(bf16)
                               .reshape(KD, 128, take))
            # chunk-major within the slot: [(chunk), k, t]
            xeT[:, KD * off:KD * (off + sz)] = np.concatenate(
                [xs[:, :, c0:c0 + cwd].transpose(1, 0, 2).reshape(128, KD * cwd)
                 for (c0, cwd) in _chunks(sz)], axis=1)
            gwb[:, off:off + take] = gw[idx, e][None, :]
            w1in[pos] = w1p[e]
            w2in[pos] = w2p[e]
            wpin[pos] = wpp[e]
            b1in[pos] = b1p[e]
            b2in[pos] = b2p[e]
            bpin[pos] = bpp[e]
            off += sz
        core_tok.append(toks)
        in_maps.append({
            "xeT": xeT, "w1": w1in, "w2": w2in, "wp": wpin,
            "b1": b1in, "b2": b2in, "bp": bpin, "gwb": gwb,
        })
    return in_maps, core_tok


def kernel(**inputs):
    from concourse.bass_utils import run_bass_kernel_spmd

    x = np.asarray(inputs["x"], dtype=np.float32)
    noise = np.asarray(inputs["noise"], dtype=np.float32)
    gate_w = np.asarray(inputs["gate_w"], dtype=np.float32)
    noise_weight = np.asarray(inputs["noise_weight"], dtype=np.float32)
    w1 = np.asarray(inputs["w1"], dtype=np.float32)
    b1 = np.asarray(inputs["b1"], dtype=np.float32)
    w2 = np.asarray(inputs["w2"], dtype=np.float32)
    b2 = np.asarray(inputs["b2"], dtype=np.float32)
    wp = np.asarray(inputs["wp"], dtype=np.float32)
    bp = np.asarray(inputs["bp"], dtype=np.float32)
    kk = int(np.asarray(inputs["k"]))

    B, S, _ = x.shape
    T = B * S
    x2d = np.ascontiguousarray(x.reshape(T, D))
    noise2d = noise.reshape(T, E)

    sel, gw = _route(x2d, noise2d, gate_w, noise_weight, kk)
    idxs = [np.nonzero(sel[:, e])[0] for e in range(E)]
    counts = [len(i) for i in idxs]
    sizes, plan = _plan(counts)

    if sizes not in _NC_CACHE:
        _NC_CACHE[sizes] = _build(sizes)
    nc = _NC_CACHE[sizes]

    in_maps, core_tok = _prep_maps(x2d, gw, idxs, sizes, plan,
                                   w1, b1, w2, b2, wp, bp)
    res = run_bass_kernel_spmd(nc, in_maps, core_ids=list(range(8))).results

    y2d = np.zeros((T, D), dtype=np.float32)
    for core in range(8):
        off = 0
        for pos, idx in enumerate(core_tok[core]):
            n = len(idx)
            if n:
                y2d[idx] += (res[core]["outT"][:, off:off + n]
                             .astype(np.float32).T)
            off += sizes[pos]
    return y2d.reshape(B, S, D)
